# revision 1
# baseline (speedup 1.0000x reference)
"""Cross-attention head on 8 TRN2 NeuronCores, data-parallel over batch.

Per core (one batch element b):
    xb  = x[b]  as [C=768, S=2304]  (natural layout of [C,H,W])
    xtb = xt[b] as [C=768, T=2304]
    QT[d,s] = sum_c wqT[c,d] xb[c,s] + bq[d]        (lhsT=wqT blk, rhs=xb)
    KT[d,t] = sum_c wkT[c,d] xtb[c,t] + bk[d]
    V[t,d]  = sum_c xtb[c,t] wvT[c,d] + bv[d]       (lhsT=xtb blk, rhs=wvT)
    E[t,s]  = exp(sum_d KT[d,t] QT[d,s] / sqrt(D))  (scoresT tiles, exp on ACT)
    den[s]  = sum_t E[t,s]                           (ones-vector matmuls)
    out[s,d]= (sum_t E[t,s] V[t,d]) / den[s]         (lhsT=E col-slice, rhs=V)

All matmuls bf16 with fp32 PSUM accumulation. exp needs no max-subtraction:
scores ~ N(0, 0.33) for these inputs, |scores| < ~4.
"""
import sys

for _p in ("/opt/trn_rl_repo",):
    if _p not in sys.path:
        sys.path.insert(0, _p)

import math

import ml_dtypes
import numpy as np

import concourse.bacc as bacc
import concourse.bass as bass
import concourse.mybir as mybir
import concourse.tile as tile
from concourse.bass_utils import run_bass_kernel_spmd

BF16 = mybir.dt.bfloat16
F32 = mybir.dt.float32

N_CORES = 8
C = 768        # input channels
S = 2304       # query positions (48*48)
T = 2304       # key positions
D = 512        # head dim
P = 128        # partitions

C_BLKS = C // P          # 6
D_BLKS = D // P          # 4
T_BLKS = T // P          # 18
S_BLKS = S // P          # 18
# free-dim slices of S for projection / scores matmuls (PSUM bank = 512 fp32)
S_SLICES = [(i * 512, min(512, S - i * 512)) for i in range((S + 511) // 512)]
INV_SQRT_D = 1.0 / math.sqrt(D)


def build_kernel(reps=1):
    nc = bacc.Bacc("TRN2", target_bir_lowering=False)

    xb_d = nc.dram_tensor("xb", [C, S], BF16, kind="ExternalInput")
    xtb_d = nc.dram_tensor("xtb", [C, T], BF16, kind="ExternalInput")
    wqt_d = nc.dram_tensor("wqt", [C, D], BF16, kind="ExternalInput")
    wkt_d = nc.dram_tensor("wkt", [C, D], BF16, kind="ExternalInput")
    wvt_d = nc.dram_tensor("wvt", [C, D], BF16, kind="ExternalInput")
    bq_d = nc.dram_tensor("bq", [1, D], F32, kind="ExternalInput")
    bk_d = nc.dram_tensor("bk", [1, D], F32, kind="ExternalInput")
    bv_d = nc.dram_tensor("bv", [1, D], BF16, kind="ExternalInput")
    out_d = nc.dram_tensor("out", [S, D], F32, kind="ExternalOutput")

    with tile.TileContext(nc) as tc:
        for _rep in range(reps):
            _emit_body(nc, tc, xb_d, xtb_d, wqt_d, wkt_d, wvt_d,
                       bq_d, bk_d, bv_d, out_d)

    nc.compile()
    return nc


def _emit_body(nc, tc, xb_d, xtb_d, wqt_d, wkt_d, wvt_d, bq_d, bk_d, bv_d,
               out_d):
    if True:
        with (
            # x tiles, xt tiles and E tiles share 18 slots of [128, 2304] bf16
            tc.tile_pool(name="big", bufs=18) as big,
            tc.tile_pool(name="wt", bufs=18) as wt,
            tc.tile_pool(name="qk", bufs=8) as qk,
            tc.tile_pool(name="vp", bufs=18) as vp,
            tc.tile_pool(name="small", bufs=1) as small,
            tc.tile_pool(name="outp", bufs=4) as outp,
        ):
            # ---- loads ----
            # Ordered so PE can start ASAP: wq first, then x in s-slice
            # chunks (QT consumes them s-major), then wk, xt, wv.
            def load_w(wname, wd, chunked=False):
                tiles = [wt.tile([P, D], BF16, tag="wt", name=f"w{wname}_{cb}")
                         for cb in range(C_BLKS)]
                if chunked:
                    # db-major chunks: the first projection group (db=0) only
                    # needs column block 0 of every c-tile
                    for db in range(D_BLKS):
                        for cb in range(C_BLKS):
                            nc.sync.dma_start(
                                out=tiles[cb][:, db * P:(db + 1) * P],
                                in_=wd[cb * P:(cb + 1) * P, db * P:(db + 1) * P])
                else:
                    for cb in range(C_BLKS):
                        nc.sync.dma_start(out=tiles[cb],
                                          in_=wd[cb * P:(cb + 1) * P, :])
                return tiles

            def load_x(xd, prefix):
                tiles = [big.tile([P, S], BF16, tag="big", name=f"{prefix}_{cb}")
                         for cb in range(C_BLKS)]
                for s0, sw in S_SLICES:
                    for cb in range(C_BLKS):
                        nc.sync.dma_start(
                            out=tiles[cb][:, s0:s0 + sw],
                            in_=xd[cb * P:(cb + 1) * P, s0:s0 + sw])
                return tiles

            # tiny tensors first so nothing downstream waits on them
            # (single-descriptor [1,D] row loads; [128,1] layouts are made
            # on-chip with trivial matmuls — a [128,1] DMA is 128 tiny
            # descriptors and costs microseconds)
            bqr = small.tile([1, D], F32, tag="bqr", name="bqr")
            nc.sync.dma_start(out=bqr, in_=bq_d[:, :])
            bkr = small.tile([1, D], F32, tag="bkr", name="bkr")
            nc.sync.dma_start(out=bkr, in_=bk_d[:, :])
            bvr = small.tile([1, D], BF16, tag="bvr", name="bvr")
            nc.sync.dma_start(out=bvr, in_=bv_d[:, :])
            ones_t = small.tile([P, 1], BF16, tag="ones", name="ones_t")
            nc.vector.memset(ones_t, 1.0)
            ones_row = small.tile([1, P], BF16, tag="ones_row", name="ones_row")
            nc.vector.memset(ones_row, 1.0)
            one11 = small.tile([1, 1], F32, tag="one11", name="one11")
            nc.vector.memset(one11, 1.0)
            bq_sb = [small.tile([P, 1], F32, tag=f"bq{db}", name=f"bq_{db}")
                     for db in range(D_BLKS)]
            bk_sb = [small.tile([P, 1], F32, tag=f"bk{db}", name=f"bk_{db}")
                     for db in range(D_BLKS)]
            bv_bc = small.tile([P, D], F32, tag="bv_bc", name="bv_bc")

            w_sb = {}
            w_sb["q"] = load_w("q", wqt_d)
            x_sb = load_x(xb_d, "x")
            w_sb["k"] = load_w("k", wkt_d)
            xt_sb = load_x(xtb_d, "xt")
            w_sb["v"] = load_w("v", wvt_d)

            # ---- phase 1: projections ----
            qt_sb = [qk.tile([P, S], BF16, tag="qk", name=f"qt_{db}")
                     for db in range(D_BLKS)]
            kt_sb = [qk.tile([P, T], BF16, tag="qk", name=f"kt_{db}")
                     for db in range(D_BLKS)]
            v_sb = [vp.tile([P, D], BF16, tag="vp", name=f"v_{tb}")
                    for tb in range(T_BLKS)]

            with tc.tile_pool(name="pp", bufs=5, space="PSUM") as pp:
                # bias prep on the otherwise-idle PE: transpose bq/bk rows to
                # [128,1] per-partition scalars; broadcast bv to [128, D]
                for db in range(D_BLKS):
                    for row, dst_t in ((bqr, bq_sb[db]), (bkr, bk_sb[db])):
                        ps_b = pp.tile([P, 1], F32, tag="ps_b",
                                       name=f"ps_b_{db}", bufs=2)
                        nc.tensor.matmul(ps_b, row[:, db * P:(db + 1) * P],
                                         one11, start=True, stop=True)
                        nc.vector.tensor_copy(dst_t, ps_b)
                ps_bv = pp.tile([P, D], F32, tag="ps_bv", name="ps_bv", bufs=1)
                nc.tensor.matmul(ps_bv, ones_row, bvr, start=True, stop=True)
                nc.vector.tensor_copy(bv_bc, ps_bv)

                for which, w_tiles, rhs_tiles, dst, bias in (
                    ("q", w_sb["q"], x_sb, qt_sb, bq_sb),
                    ("k", w_sb["k"], xt_sb, kt_sb, bk_sb),
                ):
                    for s0, sw in S_SLICES:
                        for db in range(D_BLKS):
                            ps = pp.tile([P, 512], F32, tag="pp",
                                         name=f"ps_{which}_{db}_{s0}")
                            for cb in range(C_BLKS):
                                nc.tensor.matmul(
                                    ps[:, :sw],
                                    w_tiles[cb][:, db * P:(db + 1) * P],
                                    rhs_tiles[cb][:, s0:s0 + sw],
                                    start=(cb == 0),
                                    stop=(cb == C_BLKS - 1),
                                )
                            nc.vector.tensor_scalar_add(
                                dst[db][:, s0:s0 + sw], ps[:, :sw], bias[db])
                for tb in range(T_BLKS):
                    ps = pp.tile([P, 512], F32, tag="pp", name=f"ps_v_{tb}")
                    for cb in range(C_BLKS):
                        nc.tensor.matmul(
                            ps,
                            xt_sb[cb][:, tb * P:(tb + 1) * P],
                            w_sb["v"][cb],
                            start=(cb == 0),
                            stop=(cb == C_BLKS - 1),
                        )
                    nc.vector.tensor_add(v_sb[tb], ps, bv_bc)

            # ---- phase 2: scoresT + exp + den ----
            e_sb = [big.tile([P, S], BF16, tag="big", name=f"e_{tb}")
                    for tb in range(T_BLKS)]
            den_row = small.tile([1, S], F32, tag="den_row", name="den_row")

            with (
                tc.tile_pool(name="sp", bufs=6, space="PSUM") as sp,
                tc.tile_pool(name="dp", bufs=2, space="PSUM") as dp,
            ):
                for s0, sw in S_SLICES:
                    den_ps = dp.tile([1, 512], F32, tag="dp", name=f"den_{s0}")

                    def den_mm(tb):
                        nc.tensor.matmul(
                            den_ps[:, :sw],
                            ones_t,
                            e_sb[tb][:, s0:s0 + sw],
                            start=(tb == 0),
                            stop=(tb == T_BLKS - 1),
                        )

                    for tb in range(T_BLKS):
                        ps = sp.tile([P, 512], F32, tag="sp",
                                     name=f"ps_s_{s0}_{tb}")
                        for db in range(D_BLKS):
                            nc.tensor.matmul(
                                ps[:, :sw],
                                kt_sb[db][:, tb * P:(tb + 1) * P],
                                qt_sb[db][:, s0:s0 + sw],
                                start=(db == 0),
                                stop=(db == D_BLKS - 1),
                            )
                        nc.scalar.activation(
                            e_sb[tb][:, s0:s0 + sw], ps[:, :sw],
                            mybir.ActivationFunctionType.Exp,
                            scale=INV_SQRT_D,
                        )
                        # den matmuls lag 2 tiles behind so PE never waits on ACT
                        if tb >= 2:
                            den_mm(tb - 2)
                    den_mm(T_BLKS - 2)
                    den_mm(T_BLKS - 1)
                    nc.vector.tensor_copy(den_row[:, s0:s0 + sw],
                                          den_ps[:, :sw])

            # ---- phase 3: out = (E^T @ V) * rden ----
            rden_row = small.tile([1, S], F32, tag="rden_row", name="rden_row")
            nc.vector.reciprocal(rden_row, den_row)
            rden_sb = [small.tile([P, 1], F32, tag=f"rden{sb}", name=f"rden_{sb}")
                       for sb in range(S_BLKS)]

            with (
                tc.tile_pool(name="op", bufs=4, space="PSUM") as op,
                tc.tile_pool(name="rp", bufs=2, space="PSUM") as rp,
            ):
                for sb in range(S_BLKS):
                    ops = op.tile([P, D], F32, tag="op", name=f"o_{sb}")
                    for tb in range(T_BLKS):
                        nc.tensor.matmul(
                            ops,
                            e_sb[tb][:, sb * P:(sb + 1) * P],
                            v_sb[tb],
                            start=(tb == 0),
                            stop=(tb == T_BLKS - 1),
                        )
                    if sb == 0:
                        # transpose rden [1, S] into per-block [128, 1] via
                        # trivial K=1 matmuls; runs on PE after the first PV
                        # group so den (end of phase 2) is certainly ready.
                        for sb2 in range(S_BLKS):
                            rps = rp.tile([P, 1], F32, tag="rp",
                                          name=f"rps_{sb2}")
                            nc.tensor.matmul(
                                rps,
                                rden_row[:, sb2 * P:(sb2 + 1) * P],
                                one11,
                                start=True,
                                stop=True,
                            )
                            nc.vector.tensor_copy(rden_sb[sb2], rps)
                    out_t = outp.tile([P, D], F32, tag="outp", name=f"out_{sb}")
                    nc.vector.tensor_scalar_mul(out_t, ops, rden_sb[sb])
                    nc.sync.dma_start(out=out_d[sb * P:(sb + 1) * P, :],
                                      in_=out_t)


_NC = None


def _get_nc():
    global _NC
    if _NC is None:
        _NC = build_kernel()
    return _NC


def make_in_maps(x, xt, wq, bq, wk, bk, wv, bv):
    bf = ml_dtypes.bfloat16
    wqt = np.ascontiguousarray(np.asarray(wq, np.float32).T).astype(bf)
    wkt = np.ascontiguousarray(np.asarray(wk, np.float32).T).astype(bf)
    wvt = np.ascontiguousarray(np.asarray(wv, np.float32).T).astype(bf)
    bq_h = np.ascontiguousarray(np.asarray(bq, np.float32).reshape(1, D))
    bk_h = np.ascontiguousarray(np.asarray(bk, np.float32).reshape(1, D))
    bv_h = np.asarray(bv, np.float32).reshape(1, D).astype(bf)

    in_maps = []
    for b in range(x.shape[0]):
        in_maps.append({
            "xb": np.ascontiguousarray(
                np.asarray(x[b], np.float32).reshape(C, S)).astype(bf),
            "xtb": np.ascontiguousarray(
                np.asarray(xt[b], np.float32).reshape(C, T)).astype(bf),
            "wqt": wqt, "wkt": wkt, "wvt": wvt,
            "bq": bq_h, "bk": bk_h, "bv": bv_h,
        })
    return in_maps


def kernel(x, xt, wq, bq, wk, bk, wv, bv):
    B = x.shape[0]
    assert B == N_CORES
    in_maps = make_in_maps(x, xt, wq, bq, wk, bk, wv, bv)
    nc = _get_nc()
    r = run_bass_kernel_spmd(nc, in_maps, core_ids=list(range(N_CORES)))
    return np.stack([r.results[b]["out"] for b in range(B)], axis=0)



# revision 8
# speedup vs baseline: 1.1110x; 1.1110x over previous
"""Cross-attention head on 8 TRN2 NeuronCores, data-parallel over batch.

Per core (one batch element b):
    xb  = x[b]  as [C=768, S=2304]  (natural layout of [C,H,W])
    xtb = xt[b] as [C=768, T=2304]
    QT[d,s] = sum_c wqT[c,d] xb[c,s] + bq[d]        (lhsT=wqT blk, rhs=xb)
    KT[d,t] = sum_c wkT[c,d] xtb[c,t] + bk[d]
    V[t,d]  = sum_c xtb[c,t] wvT[c,d] + bv[d]       (lhsT=xtb blk, rhs=wvT)
    E[t,s]  = exp(sum_d KT[d,t] QT[d,s] / sqrt(D))  (scoresT tiles, exp on ACT)
    esum[p,s] += E[p+128*tb, s]  on DVE (f32)        (den partial sums)
    den_bc[p,s] = sum_p' esum[p',s]                  (ones[128,128] f32r matmul)
    outT[d,s] = (sum_t V[t,d] E[t,s]) * rden_bc[:,s] (lhsT=V d-slice, rhs=E)
Host transposes outT -> [S, D].

Matmuls are ordered so consecutive instructions share the stationary
operand (lhsT) wherever possible — s-slices innermost — to amortize
PE weight loads. The softmax denominator never runs on the PE as a
ones-vector reduction over T (that cost 41k columns + 108 matmuls in v1);
it is accumulated on the DVE and collapsed by five 512-column f32r
matmuls instead.

All matmuls bf16 with fp32 PSUM accumulation. exp needs no max-subtraction:
scores ~ N(0, 0.33) for these inputs, |scores| < ~4.
"""
import sys

for _p in ("/opt/trn_rl_repo",):
    if _p not in sys.path:
        sys.path.insert(0, _p)

import math

import ml_dtypes
import numpy as np

import concourse.bacc as bacc
import concourse.bass as bass
import concourse.mybir as mybir
import concourse.tile as tile
from concourse.bass_utils import run_bass_kernel_spmd

BF16 = mybir.dt.bfloat16
F32 = mybir.dt.float32
F32R = mybir.dt.float32r

N_CORES = 8
C = 768        # input channels
S = 2304       # query positions (48*48)
T = 2304       # key positions
D = 512        # head dim
P = 128        # partitions

C_BLKS = C // P          # 6
D_BLKS = D // P          # 4
T_BLKS = T // P          # 18
S_BLKS = S // P          # 18
# free-dim slices of S for projection / scores / PV matmuls (PSUM bank =
# 512 fp32)
S_SLICES = [(i * 512, min(512, S - i * 512)) for i in range((S + 511) // 512)]
INV_SQRT_D = 1.0 / math.sqrt(D)


def build_kernel(reps=1):
    nc = bacc.Bacc("TRN2", target_bir_lowering=False)

    xb_d = nc.dram_tensor("xb", [C, S], BF16, kind="ExternalInput")
    xtb_d = nc.dram_tensor("xtb", [C, T], BF16, kind="ExternalInput")
    wqt_d = nc.dram_tensor("wqt", [C, D], BF16, kind="ExternalInput")
    wkt_d = nc.dram_tensor("wkt", [C, D], BF16, kind="ExternalInput")
    wvt_d = nc.dram_tensor("wvt", [C, D], BF16, kind="ExternalInput")
    bq_d = nc.dram_tensor("bq", [1, D], F32, kind="ExternalInput")
    bk_d = nc.dram_tensor("bk", [1, D], F32, kind="ExternalInput")
    bv_d = nc.dram_tensor("bv", [1, D], BF16, kind="ExternalInput")
    outt_d = nc.dram_tensor("outT", [D, S], F32, kind="ExternalOutput")

    with tile.TileContext(nc) as tc:
        for _rep in range(reps):
            _emit_body(nc, tc, xb_d, xtb_d, wqt_d, wkt_d, wvt_d,
                       bq_d, bk_d, bv_d, outt_d)

    nc.compile()
    return nc


def _emit_body(nc, tc, xb_d, xtb_d, wqt_d, wkt_d, wvt_d, bq_d, bk_d, bv_d,
               outt_d):
    with (
        # x tiles, xt tiles and E tiles share 18 slots of [128, 2304] bf16
        tc.tile_pool(name="big", bufs=18) as big,
        tc.tile_pool(name="wt", bufs=18) as wt,
        tc.tile_pool(name="qk", bufs=8) as qk,
        tc.tile_pool(name="vp", bufs=18) as vp,
        tc.tile_pool(name="small", bufs=1) as small,
        tc.tile_pool(name="outp", bufs=4) as outp,
    ):
        # ---- loads ----
        # Ordered so PE can start ASAP: wq db-chunked first (the db=0
        # projection group only needs column block 0 of every c-tile),
        # then x in s-slice chunks, then wk, xt, wv.
        def load_w(wname, wd, chunked=False):
            tiles = [wt.tile([P, D], BF16, tag="wt", name=f"w{wname}_{cb}")
                     for cb in range(C_BLKS)]
            if chunked:
                for db in range(D_BLKS):
                    for cb in range(C_BLKS):
                        nc.sync.dma_start(
                            out=tiles[cb][:, db * P:(db + 1) * P],
                            in_=wd[cb * P:(cb + 1) * P, db * P:(db + 1) * P])
            else:
                for cb in range(C_BLKS):
                    nc.sync.dma_start(out=tiles[cb],
                                      in_=wd[cb * P:(cb + 1) * P, :])
            return tiles

        def load_x(xd, prefix):
            tiles = [big.tile([P, S], BF16, tag="big", name=f"{prefix}_{cb}")
                     for cb in range(C_BLKS)]
            for s0, sw in S_SLICES:
                for cb in range(C_BLKS):
                    nc.sync.dma_start(
                        out=tiles[cb][:, s0:s0 + sw],
                        in_=xd[cb * P:(cb + 1) * P, s0:s0 + sw])
            return tiles

        # tiny tensors first so nothing downstream waits on them
        bqr = small.tile([1, D], F32, tag="bqr", name="bqr")
        nc.sync.dma_start(out=bqr, in_=bq_d[:, :])
        bkr = small.tile([1, D], F32, tag="bkr", name="bkr")
        nc.sync.dma_start(out=bkr, in_=bk_d[:, :])
        bvr = small.tile([1, D], BF16, tag="bvr", name="bvr")
        nc.sync.dma_start(out=bvr, in_=bv_d[:, :])
        ones_row = small.tile([1, P], BF16, tag="ones_row", name="ones_row")
        nc.vector.memset(ones_row, 1.0)
        one11 = small.tile([1, 1], F32, tag="one11", name="one11")
        nc.vector.memset(one11, 1.0)
        ones_mat_f = small.tile([P, P], F32, tag="ones_mat_f",
                                name="ones_mat_f")
        nc.vector.memset(ones_mat_f, 1.0)
        ones_mat = small.tile([P, P], F32R, tag="ones_mat", name="ones_mat")
        nc.vector.tensor_copy(ones_mat, ones_mat_f)
        bq_sb = [small.tile([P, 1], F32, tag=f"bq{db}", name=f"bq_{db}")
                 for db in range(D_BLKS)]
        bk_sb = [small.tile([P, 1], F32, tag=f"bk{db}", name=f"bk_{db}")
                 for db in range(D_BLKS)]
        bv_bc = small.tile([P, D], F32, tag="bv_bc", name="bv_bc")
        esum = small.tile([P, S], F32R, tag="esum", name="esum")
        rden_bc = small.tile([P, S], F32, tag="rden_bc", name="rden_bc")

        w_sb = {}
        w_sb["q"] = load_w("q", wqt_d, chunked=True)
        x_sb = load_x(xb_d, "x")
        w_sb["k"] = load_w("k", wkt_d, chunked=True)
        xt_sb = load_x(xtb_d, "xt")
        w_sb["v"] = load_w("v", wvt_d)

        # ---- phase 1: projections ----
        qt_sb = [qk.tile([P, S], BF16, tag="qk", name=f"qt_{db}")
                 for db in range(D_BLKS)]
        kt_sb = [qk.tile([P, T], BF16, tag="qk", name=f"kt_{db}")
                 for db in range(D_BLKS)]
        v_sb = [vp.tile([P, D], BF16, tag="vp", name=f"v_{tb}")
                for tb in range(T_BLKS)]

        with tc.tile_pool(name="pp", bufs=5, space="PSUM") as pp:
            # bias prep on the otherwise-idle PE: transpose bq/bk rows to
            # [128,1] per-partition scalars; broadcast bv to [128, D]
            for db in range(D_BLKS):
                for row, dst_t in ((bqr, bq_sb[db]), (bkr, bk_sb[db])):
                    ps_b = pp.tile([P, 1], F32, tag="ps_b",
                                   name=f"ps_b_{db}", bufs=2)
                    nc.tensor.matmul(ps_b, row[:, db * P:(db + 1) * P],
                                     one11, start=True, stop=True)
                    nc.vector.tensor_copy(dst_t, ps_b)
            ps_bv = pp.tile([P, D], F32, tag="ps_bv", name="ps_bv", bufs=1)
            nc.tensor.matmul(ps_bv, ones_row, bvr, start=True, stop=True)
            nc.vector.tensor_copy(bv_bc, ps_bv)

            # Q/K: db outer, cb middle, s-slice inner — 5 matmuls per
            # weight load, 5 live PSUM banks per db group.
            for which, w_tiles, rhs_tiles, dst, bias in (
                ("q", w_sb["q"], x_sb, qt_sb, bq_sb),
                ("k", w_sb["k"], xt_sb, kt_sb, bk_sb),
            ):
                for db in range(D_BLKS):
                    ps = [pp.tile([P, 512], F32, tag="pp",
                                  name=f"ps_{which}_{db}_{si}")
                          for si in range(len(S_SLICES))]
                    for cb in range(C_BLKS):
                        lhsT = w_tiles[cb][:, db * P:(db + 1) * P]
                        for si, (s0, sw) in enumerate(S_SLICES):
                            nc.tensor.matmul(
                                ps[si][:, :sw],
                                lhsT,
                                rhs_tiles[cb][:, s0:s0 + sw],
                                start=(cb == 0),
                                stop=(cb == C_BLKS - 1),
                            )
                    for si, (s0, sw) in enumerate(S_SLICES):
                        nc.vector.tensor_scalar_add(
                            dst[db][:, s0:s0 + sw], ps[si][:, :sw], bias[db])
            # V: per t-block, accumulate over cb (no lhsT reuse available)
            for tb in range(T_BLKS):
                ps = pp.tile([P, 512], F32, tag="pp", name=f"ps_v_{tb}")
                for cb in range(C_BLKS):
                    nc.tensor.matmul(
                        ps,
                        xt_sb[cb][:, tb * P:(tb + 1) * P],
                        w_sb["v"][cb],
                        start=(cb == 0),
                        stop=(cb == C_BLKS - 1),
                    )
                nc.vector.tensor_add(v_sb[tb], ps, bv_bc)

        # ---- phase 2: scoresT + exp + den partial sums ----
        e_sb = [big.tile([P, S], BF16, tag="big", name=f"e_{tb}")
                for tb in range(T_BLKS)]

        with tc.tile_pool(name="sp", bufs=7, space="PSUM") as sp:
            for tb in range(T_BLKS):
                ps = [sp.tile([P, 512], F32, tag="sp",
                              name=f"ps_s_{tb}_{si}")
                      for si in range(len(S_SLICES))]
                for db in range(D_BLKS):
                    lhsT = kt_sb[db][:, tb * P:(tb + 1) * P]
                    for si, (s0, sw) in enumerate(S_SLICES):
                        nc.tensor.matmul(
                            ps[si][:, :sw],
                            lhsT,
                            qt_sb[db][:, s0:s0 + sw],
                            start=(db == 0),
                            stop=(db == D_BLKS - 1),
                        )
                for si, (s0, sw) in enumerate(S_SLICES):
                    nc.scalar.activation(
                        e_sb[tb][:, s0:s0 + sw], ps[si][:, :sw],
                        mybir.ActivationFunctionType.Exp,
                        scale=INV_SQRT_D,
                    )
                    if tb == 0:
                        nc.vector.tensor_copy(esum[:, s0:s0 + sw],
                                              e_sb[tb][:, s0:s0 + sw])
                    else:
                        nc.vector.tensor_add(esum[:, s0:s0 + sw],
                                             esum[:, s0:s0 + sw],
                                             e_sb[tb][:, s0:s0 + sw])

        # ---- phase 3: outT[d,s] = (sum_t V[t,d] E[t,s]) * rden_bc ----
        # den collapse + reciprocal-broadcast matmuls are emitted after the
        # first PV accumulation group so the PE never waits on the DVE's
        # esum chain; rden_bc is only read by the DVE when draining that
        # first group, ~20us later.
        with (
            tc.tile_pool(name="op", bufs=6, space="PSUM") as op,
            tc.tile_pool(name="bc", bufs=2, space="PSUM") as bc,
        ):
            for db in range(D_BLKS):
                ps = [op.tile([P, 512], F32, tag="op",
                              name=f"o_{db}_{si}")
                      for si in range(len(S_SLICES))]
                for tb in range(T_BLKS):
                    lhsT = v_sb[tb][:, db * P:(db + 1) * P]
                    for si, (s0, sw) in enumerate(S_SLICES):
                        nc.tensor.matmul(
                            ps[si][:, :sw],
                            lhsT,
                            e_sb[tb][:, s0:s0 + sw],
                            start=(tb == 0),
                            stop=(tb == T_BLKS - 1),
                        )
                if db == 0:
                    # den_bc[p, s] = sum_p' esum[p', s] via ones f32r
                    # matmuls (1 cycle/row at N>=256), then reciprocal.
                    for si, (s0, sw) in enumerate(S_SLICES):
                        bc_ps = bc.tile([P, 512], F32, tag="bc",
                                        name=f"bc_{si}")
                        nc.tensor.matmul(
                            bc_ps[:, :sw],
                            ones_mat,
                            esum[:, s0:s0 + sw],
                            start=True, stop=True,
                        )
                        nc.vector.reciprocal(rden_bc[:, s0:s0 + sw],
                                             bc_ps[:, :sw])
                for si, (s0, sw) in enumerate(S_SLICES):
                    out_t = outp.tile([P, 512], F32, tag="outp",
                                      name=f"out_{db}_{si}")
                    nc.vector.tensor_mul(out_t[:, :sw], ps[si][:, :sw],
                                         rden_bc[:, s0:s0 + sw])
                    nc.sync.dma_start(
                        out=outt_d[db * P:(db + 1) * P, s0:s0 + sw],
                        in_=out_t[:, :sw])


_NC = None


def _get_nc():
    global _NC
    if _NC is None:
        _NC = build_kernel()
    return _NC


def make_in_maps(x, xt, wq, bq, wk, bk, wv, bv):
    bf = ml_dtypes.bfloat16
    wqt = np.ascontiguousarray(np.asarray(wq, np.float32).T).astype(bf)
    wkt = np.ascontiguousarray(np.asarray(wk, np.float32).T).astype(bf)
    wvt = np.ascontiguousarray(np.asarray(wv, np.float32).T).astype(bf)
    bq_h = np.ascontiguousarray(np.asarray(bq, np.float32).reshape(1, D))
    bk_h = np.ascontiguousarray(np.asarray(bk, np.float32).reshape(1, D))
    bv_h = np.asarray(bv, np.float32).reshape(1, D).astype(bf)

    in_maps = []
    for b in range(x.shape[0]):
        in_maps.append({
            "xb": np.ascontiguousarray(
                np.asarray(x[b], np.float32).reshape(C, S)).astype(bf),
            "xtb": np.ascontiguousarray(
                np.asarray(xt[b], np.float32).reshape(C, T)).astype(bf),
            "wqt": wqt, "wkt": wkt, "wvt": wvt,
            "bq": bq_h, "bk": bk_h, "bv": bv_h,
        })
    return in_maps


def kernel(x, xt, wq, bq, wk, bk, wv, bv):
    B = x.shape[0]
    assert B == N_CORES
    in_maps = make_in_maps(x, xt, wq, bq, wk, bk, wv, bv)
    nc = _get_nc()
    r = run_bass_kernel_spmd(nc, in_maps, core_ids=list(range(N_CORES)))
    return np.stack([np.ascontiguousarray(r.results[b]["outT"].T)
                     for b in range(B)], axis=0)


# revision 16
# speedup vs baseline: 1.1818x; 1.0638x over previous
"""Cross-attention head on 8 TRN2 NeuronCores, data-parallel over batch.

Per core (one batch element b):
    xb  = x[b]  as [C=768, S=2304]  (natural layout of [C,H,W])
    xtb = xt[b] as [C=768, T=2304]
    QT[d,s] = sum_c wqT[c,d] xb[c,s] + bq[d]        (lhsT=wqT blk, rhs=xb)
    KT[d,t] = sum_c wkT[c,d] xtb[c,t] + bk[d]
    V[t,d]  = sum_c xtb[c,t] wvT[c,d] + bv[d]       (lhsT=xtb blk, rhs=wvT)
    E[t,s]  = exp(sum_d KT[d,t] QT[d,s] / sqrt(D))  (scoresT tiles, exp on ACT)
    esum[p,s] += E[p+128*tb, s]  on DVE (f32)        (den partial sums)
    den_bc[p,s] = sum_p' esum[p',s]                  (ones[128,128] f32r matmul)
    outT[d,s] = (sum_t V[t,d] E[t,s]) * rden_bc[:,s] (lhsT=V d-slice, rhs=E)
Host transposes outT -> [S, D].

Matmuls are ordered so consecutive instructions share the stationary
operand (lhsT) wherever possible — s-slices innermost — to amortize
PE weight loads. The softmax denominator never runs on the PE as a
ones-vector reduction over T (that cost 41k columns + 108 matmuls in v1);
it is accumulated on the DVE and collapsed by five 512-column f32r
matmuls instead.

All matmuls bf16 with fp32 PSUM accumulation. exp needs no max-subtraction:
scores ~ N(0, 0.33) for these inputs, |scores| < ~4.
"""
import sys

for _p in ("/opt/trn_rl_repo",):
    if _p not in sys.path:
        sys.path.insert(0, _p)

import math

import ml_dtypes
import numpy as np

import concourse.bacc as bacc
import concourse.bass as bass
import concourse.mybir as mybir
import concourse.tile as tile
from concourse.bass_utils import run_bass_kernel_spmd

BF16 = mybir.dt.bfloat16
F32 = mybir.dt.float32
F32R = mybir.dt.float32r

N_CORES = 8
C = 768        # input channels
S = 2304       # query positions (48*48)
T = 2304       # key positions
D = 512        # head dim
P = 128        # partitions

C_BLKS = C // P          # 6
D_BLKS = D // P          # 4
T_BLKS = T // P          # 18
S_BLKS = S // P          # 18
# free-dim slices of S for projection / scores / PV matmuls (PSUM bank =
# 512 fp32)
S_SLICES = [(i * 512, min(512, S - i * 512)) for i in range((S + 511) // 512)]
INV_SQRT_D = 1.0 / math.sqrt(D)


def build_kernel(reps=1):
    nc = bacc.Bacc("TRN2", target_bir_lowering=False)

    xb_d = nc.dram_tensor("xb", [C, S], BF16, kind="ExternalInput")
    xtb_d = nc.dram_tensor("xtb", [C, T], BF16, kind="ExternalInput")
    wqt_d = nc.dram_tensor("wqt", [C, D], BF16, kind="ExternalInput")
    wkt_d = nc.dram_tensor("wkt", [C, D], BF16, kind="ExternalInput")
    wvt_d = nc.dram_tensor("wvt", [C, D], BF16, kind="ExternalInput")
    bq_d = nc.dram_tensor("bq", [1, D], F32, kind="ExternalInput")
    bk_d = nc.dram_tensor("bk", [1, D], F32, kind="ExternalInput")
    bv_d = nc.dram_tensor("bv", [1, D], BF16, kind="ExternalInput")
    outt_d = nc.dram_tensor("outT", [D, S], F32, kind="ExternalOutput")

    with tile.TileContext(nc) as tc:
        for _rep in range(reps):
            _emit_body(nc, tc, xb_d, xtb_d, wqt_d, wkt_d, wvt_d,
                       bq_d, bk_d, bv_d, outt_d)

    nc.compile()
    return nc


def _emit_body(nc, tc, xb_d, xtb_d, wqt_d, wkt_d, wvt_d, bq_d, bk_d, bv_d,
               outt_d):
    with (
        # x tiles and E tiles share 18 slots of [128, 2304] bf16; xt gets
        # its own pool so the next rep's xt DMA (feeding its V-proj, the
        # first PE phase) can prefetch as soon as this rep's K-proj is done
        # instead of waiting for the final PV reads of the E tiles.
        tc.tile_pool(name="big", bufs=18) as big,
        tc.tile_pool(name="xtp", bufs=6) as xtp,
        tc.tile_pool(name="wt", bufs=12) as wt,
        tc.tile_pool(name="qk", bufs=8) as qk,
        tc.tile_pool(name="vp", bufs=18) as vp,
        tc.tile_pool(name="small", bufs=1) as small,
        tc.tile_pool(name="outp", bufs=2) as outp,
    ):
        # ---- loads ----
        # Ordered so PE can start ASAP: wq db-chunked first (the db=0
        # projection group only needs column block 0 of every c-tile),
        # then x in s-slice chunks, then wk, xt, wv.
        def load_w(wname, wd, chunked=False):
            tiles = [wt.tile([P, D], BF16, tag="wt", name=f"w{wname}_{cb}")
                     for cb in range(C_BLKS)]
            if chunked:
                for db in range(D_BLKS):
                    for cb in range(C_BLKS):
                        nc.sync.dma_start(
                            out=tiles[cb][:, db * P:(db + 1) * P],
                            in_=wd[cb * P:(cb + 1) * P, db * P:(db + 1) * P])
            else:
                for cb in range(C_BLKS):
                    nc.sync.dma_start(out=tiles[cb],
                                      in_=wd[cb * P:(cb + 1) * P, :])
            return tiles

        def load_x(xd, prefix, pool=None, tag="big"):
            pool = big if pool is None else pool
            tiles = [pool.tile([P, S], BF16, tag=tag, name=f"{prefix}_{cb}")
                     for cb in range(C_BLKS)]
            for s0, sw in S_SLICES:
                for cb in range(C_BLKS):
                    nc.sync.dma_start(
                        out=tiles[cb][:, s0:s0 + sw],
                        in_=xd[cb * P:(cb + 1) * P, s0:s0 + sw])
            return tiles

        # tiny tensors first so nothing downstream waits on them
        bqr = small.tile([1, D], F32, tag="bqr", name="bqr")
        nc.sync.dma_start(out=bqr, in_=bq_d[:, :])
        bkr = small.tile([1, D], F32, tag="bkr", name="bkr")
        nc.sync.dma_start(out=bkr, in_=bk_d[:, :])
        bvr = small.tile([1, D], BF16, tag="bvr", name="bvr")
        nc.sync.dma_start(out=bvr, in_=bv_d[:, :])
        ones_row = small.tile([1, P], BF16, tag="ones_row", name="ones_row")
        nc.vector.memset(ones_row, 1.0)
        one11 = small.tile([1, 1], F32, tag="one11", name="one11")
        nc.vector.memset(one11, 1.0)
        ones_mat_f = small.tile([P, P], F32, tag="ones_mat_f",
                                name="ones_mat_f")
        nc.vector.memset(ones_mat_f, 1.0)
        ones_mat = small.tile([P, P], F32R, tag="ones_mat", name="ones_mat")
        nc.vector.tensor_copy(ones_mat, ones_mat_f)
        bq_sb = [small.tile([P, 1], F32, tag=f"bq{db}", name=f"bq_{db}")
                 for db in range(D_BLKS)]
        bk_sb = [small.tile([P, 1], F32, tag=f"bk{db}", name=f"bk_{db}")
                 for db in range(D_BLKS)]
        bv_bc = small.tile([P, D], F32, tag="bv_bc", name="bv_bc")
        esum = small.tile([P, S], F32R, tag="esum", name="esum")
        rden_bc = small.tile([P, S], F32, tag="rden_bc", name="rden_bc")

        # DMA order drives PE pacing: V-proj consumes xt t-slice-major as it
        # arrives, then K and Q projections run from resident tiles while x
        # streams in behind xt.
        w_sb = {}
        w_sb["v"] = load_w("v", wvt_d)
        xt_sb = load_x(xtb_d, "xt", pool=xtp, tag="xtp")
        w_sb["k"] = load_w("k", wkt_d, chunked=True)
        x_sb = load_x(xb_d, "x")
        w_sb["q"] = load_w("q", wqt_d, chunked=True)

        # ---- phase 1: projections ----
        qt_sb = [qk.tile([P, S], BF16, tag="qk", name=f"qt_{db}")
                 for db in range(D_BLKS)]
        kt_sb = [qk.tile([P, T], BF16, tag="qk", name=f"kt_{db}")
                 for db in range(D_BLKS)]
        v_sb = [vp.tile([P, D], BF16, tag="vp", name=f"v_{tb}")
                for tb in range(T_BLKS)]

        with tc.tile_pool(name="pp", bufs=5, space="PSUM") as pp:
            # bias prep on the otherwise-idle PE: transpose bq/bk rows to
            # [128,1] per-partition scalars; broadcast bv to [128, D]
            for db in range(D_BLKS):
                for row, dst_t in ((bqr, bq_sb[db]), (bkr, bk_sb[db])):
                    ps_b = pp.tile([P, 1], F32, tag="ps_b",
                                   name=f"ps_b_{db}", bufs=2)
                    nc.tensor.matmul(ps_b, row[:, db * P:(db + 1) * P],
                                     one11, start=True, stop=True)
                    nc.vector.tensor_copy(dst_t, ps_b)
            ps_bv = pp.tile([P, D], F32, tag="ps_bv", name="ps_bv", bufs=1)
            nc.tensor.matmul(ps_bv, ones_row, bvr, start=True, stop=True)
            nc.vector.tensor_copy(bv_bc, ps_bv)

            # V first: per t-block, accumulate over cb — consumption order
            # matches xt's DMA arrival order, so the weight loads hide
            # inside the DMA pacing.
            for tb in range(T_BLKS):
                ps = pp.tile([P, 512], F32, tag="pp", name=f"ps_v_{tb}")
                for cb in range(C_BLKS):
                    nc.tensor.matmul(
                        ps,
                        xt_sb[cb][:, tb * P:(tb + 1) * P],
                        w_sb["v"][cb],
                        start=(cb == 0),
                        stop=(cb == C_BLKS - 1),
                    )
                nc.vector.tensor_add(v_sb[tb], ps, bv_bc)
            # K then Q: db outer, cb middle, s-slice inner — 5 matmuls per
            # weight load, 5 live PSUM banks per db group. Bias adds go to
            # the otherwise-idle ACT engine.
            for which, w_tiles, rhs_tiles, dst, bias in (
                ("k", w_sb["k"], xt_sb, kt_sb, bk_sb),
                ("q", w_sb["q"], x_sb, qt_sb, bq_sb),
            ):
                for db in range(D_BLKS):
                    ps = [pp.tile([P, 512], F32, tag="pp",
                                  name=f"ps_{which}_{db}_{si}")
                          for si in range(len(S_SLICES))]
                    for cb in range(C_BLKS):
                        lhsT = w_tiles[cb][:, db * P:(db + 1) * P]
                        for si, (s0, sw) in enumerate(S_SLICES):
                            nc.tensor.matmul(
                                ps[si][:, :sw],
                                lhsT,
                                rhs_tiles[cb][:, s0:s0 + sw],
                                start=(cb == 0),
                                stop=(cb == C_BLKS - 1),
                            )
                    for si, (s0, sw) in enumerate(S_SLICES):
                        nc.scalar.add(
                            dst[db][:, s0:s0 + sw], ps[si][:, :sw], bias[db])

        # ---- phase 2: scoresT + exp + den partial sums ----
        e_sb = [big.tile([P, S], BF16, tag="big", name=f"e_{tb}")
                for tb in range(T_BLKS)]

        with tc.tile_pool(name="sp", bufs=7, space="PSUM") as sp:
            for tb in range(T_BLKS):
                ps = [sp.tile([P, 512], F32, tag="sp",
                              name=f"ps_s_{tb}_{si}")
                      for si in range(len(S_SLICES))]
                for db in range(D_BLKS):
                    lhsT = kt_sb[db][:, tb * P:(tb + 1) * P]
                    for si, (s0, sw) in enumerate(S_SLICES):
                        nc.tensor.matmul(
                            ps[si][:, :sw],
                            lhsT,
                            qt_sb[db][:, s0:s0 + sw],
                            start=(db == 0),
                            stop=(db == D_BLKS - 1),
                        )
                for si, (s0, sw) in enumerate(S_SLICES):
                    nc.scalar.activation(
                        e_sb[tb][:, s0:s0 + sw], ps[si][:, :sw],
                        mybir.ActivationFunctionType.Exp,
                        scale=INV_SQRT_D,
                    )
                    if tb == 0:
                        nc.vector.tensor_copy(esum[:, s0:s0 + sw],
                                              e_sb[tb][:, s0:s0 + sw])
                    else:
                        nc.vector.tensor_add(esum[:, s0:s0 + sw],
                                             esum[:, s0:s0 + sw],
                                             e_sb[tb][:, s0:s0 + sw])

        # ---- phase 3: outT[d,s] = (sum_t V[t,d] E[t,s]) * rden_bc ----
        # den collapse + reciprocal-broadcast matmuls are emitted after the
        # first PV accumulation group so the PE never waits on the DVE's
        # esum chain; rden_bc is only read by the DVE when draining that
        # first group, ~20us later.
        with (
            tc.tile_pool(name="op", bufs=6, space="PSUM") as op,
            tc.tile_pool(name="bc", bufs=2, space="PSUM") as bc,
        ):
            for db in range(D_BLKS):
                ps = [op.tile([P, 512], F32, tag="op",
                              name=f"o_{db}_{si}")
                      for si in range(len(S_SLICES))]
                for tb in range(T_BLKS):
                    lhsT = v_sb[tb][:, db * P:(db + 1) * P]
                    for si, (s0, sw) in enumerate(S_SLICES):
                        nc.tensor.matmul(
                            ps[si][:, :sw],
                            lhsT,
                            e_sb[tb][:, s0:s0 + sw],
                            start=(tb == 0),
                            stop=(tb == T_BLKS - 1),
                        )
                    if db == 0 and 4 <= tb <= 8:
                        # den_bc[p, s] = sum_p' esum[p', s] via ones f32r
                        # matmuls (1 cycle/row at N>=256), then reciprocal.
                        # Spread one slice per tb group so the reciprocal
                        # overlaps the next group's matmuls; all done long
                        # before the db=0 drain needs rden_bc.
                        si = tb - 4
                        s0, sw = S_SLICES[si]
                        bc_ps = bc.tile([P, 512], F32, tag="bc",
                                        name=f"bc_{si}")
                        nc.tensor.matmul(
                            bc_ps[:, :sw],
                            ones_mat,
                            esum[:, s0:s0 + sw],
                            start=True, stop=True,
                        )
                        nc.vector.reciprocal(rden_bc[:, s0:s0 + sw],
                                             bc_ps[:, :sw])
                for si, (s0, sw) in enumerate(S_SLICES):
                    out_t = outp.tile([P, 512], F32, tag="outp",
                                      name=f"out_{db}_{si}")
                    nc.vector.tensor_mul(out_t[:, :sw], ps[si][:, :sw],
                                         rden_bc[:, s0:s0 + sw])
                    nc.sync.dma_start(
                        out=outt_d[db * P:(db + 1) * P, s0:s0 + sw],
                        in_=out_t[:, :sw])


_NC = None


def _get_nc():
    global _NC
    if _NC is None:
        _NC = build_kernel()
    return _NC


def make_in_maps(x, xt, wq, bq, wk, bk, wv, bv):
    bf = ml_dtypes.bfloat16
    wqt = np.ascontiguousarray(np.asarray(wq, np.float32).T).astype(bf)
    wkt = np.ascontiguousarray(np.asarray(wk, np.float32).T).astype(bf)
    wvt = np.ascontiguousarray(np.asarray(wv, np.float32).T).astype(bf)
    bq_h = np.ascontiguousarray(np.asarray(bq, np.float32).reshape(1, D))
    bk_h = np.ascontiguousarray(np.asarray(bk, np.float32).reshape(1, D))
    bv_h = np.asarray(bv, np.float32).reshape(1, D).astype(bf)

    in_maps = []
    for b in range(x.shape[0]):
        in_maps.append({
            "xb": np.ascontiguousarray(
                np.asarray(x[b], np.float32).reshape(C, S)).astype(bf),
            "xtb": np.ascontiguousarray(
                np.asarray(xt[b], np.float32).reshape(C, T)).astype(bf),
            "wqt": wqt, "wkt": wkt, "wvt": wvt,
            "bq": bq_h, "bk": bk_h, "bv": bv_h,
        })
    return in_maps


def kernel(x, xt, wq, bq, wk, bk, wv, bv):
    B = x.shape[0]
    assert B == N_CORES
    in_maps = make_in_maps(x, xt, wq, bq, wk, bk, wv, bv)
    nc = _get_nc()
    r = run_bass_kernel_spmd(nc, in_maps, core_ids=list(range(N_CORES)))
    return np.stack([np.ascontiguousarray(r.results[b]["outT"].T)
                     for b in range(B)], axis=0)
